# revision 23
# baseline (speedup 1.0000x reference)
import sys

sys.path.insert(0, "/opt/trn_rl_repo")

import numpy as np
import ml_dtypes

import concourse.bass as bass
import concourse.tile as tile
import concourse.mybir as mybir
from concourse import bacc
from concourse.bass_utils import run_bass_kernel_spmd

B, H, NI, NQ = 4096, 1024, 4096, 2048
NCORES = 8
BS = B // NCORES      # 512 batch rows per core
K = H + NI            # 5120 contraction dim
KT = K // 128         # 40 k-tiles
BT = BS // 128        # 4 batch subtiles per core
HH = H // 512         # 2 h-half passes

F32 = mybir.dt.float32
BF16 = mybir.dt.bfloat16
OP = mybir.AluOpType

_built = None
LAST_RESULT = None


def _build():
    global _built
    if _built is not None:
        return _built

    nc = bacc.Bacc()
    aht_d = nc.dram_tensor("aht", [K, BS], BF16, kind="ExternalInput")
    alt_d = nc.dram_tensor("alt", [K, BS], BF16, kind="ExternalInput")
    wht_d = nc.dram_tensor("wht", [K, H], BF16, kind="ExternalInput")
    wlt_d = nc.dram_tensor("wlt", [K, H], BF16, kind="ExternalInput")
    # z stored SBUF-layout: z_d[p, pass*2048 + bt*512 + j] = z[bt*128+p, pass*512+j]
    z_d = nc.dram_tensor("z_out", [128, HH * BT * 512], F32, kind="ExternalOutput")

    from contextlib import ExitStack

    with tile.TileContext(nc) as tc, ExitStack() as ctx:
        wpool = ctx.enter_context(tc.tile_pool(name="wpool", bufs=4))
        apool = ctx.enter_context(tc.tile_pool(name="apool", bufs=4))
        epool = ctx.enter_context(tc.tile_pool(name="epool", bufs=1))
        pspool = ctx.enter_context(tc.tile_pool(name="pspool", bufs=2, space="PSUM"))

        psums_p = {}
        zf_p = {}

        def epilogue(p):
            psums = psums_p[p]
            zf = epool.tile([128, BT * 512], F32, tag=f"zf{p}", name=f"zf{p}")
            zf_p[p] = zf
            # PSUM->SBUF copies split ACT{0,2} / Vector{1,3} so the last
            # bank's copy starts the moment its stop-matmul retires
            nc.scalar.copy(zf[:, 0:512], psums[0])
            nc.vector.tensor_scalar_add(zf[:, 512:1024], psums[1], 0.0)
            nc.scalar.copy(zf[:, 1024:1536], psums[2])
            nc.vector.tensor_scalar_add(zf[:, 1536:2048], psums[3], 0.0)

        for p in range(HH):
            hs = slice(p * 512, (p + 1) * 512)
            psums = [
                pspool.tile([128, 512], F32, tag=f"ps{i}", name=f"ps{p}_{i}")
                for i in range(BT)
            ]
            psums_p[p] = psums
            for kc in range(KT):
                ks = slice(kc * 128, (kc + 1) * 128)
                # W halves stream on SP queue, A tiles on ACT queue
                wh = wpool.tile([128, 512], BF16, tag="wh", name="wh")
                nc.sync.dma_start(out=wh, in_=wht_d[ks, hs])
                ah = apool.tile([128, BS], BF16, tag="ah", name="ah")
                nc.scalar.dma_start(out=ah, in_=aht_d[ks, :])
                wl = wpool.tile([128, 512], BF16, tag="wl", name="wl")
                nc.sync.dma_start(out=wl, in_=wlt_d[ks, hs])
                al = apool.tile([128, BS], BF16, tag="al", name="al")
                nc.scalar.dma_start(out=al, in_=alt_d[ks, :])

                if p == 1:
                    # pass-0 epilogue + writeback hidden under pass-1 GEMM
                    if kc == 1:
                        epilogue(0)
                    if kc == 3:
                        nc.sync.dma_start(out=z_d[:, 0:2048], in_=zf_p[0])

                start = kc == 0
                stop = kc == KT - 1
                for bt_i in range(BT):
                    ah_s = ah[:, bt_i * 128 : (bt_i + 1) * 128]
                    al_s = al[:, bt_i * 128 : (bt_i + 1) * 128]
                    nc.tensor.matmul(psums[bt_i], ah_s, wh, start=start, stop=False)
                    nc.tensor.matmul(psums[bt_i], ah_s, wl, start=False, stop=False)
                    nc.tensor.matmul(psums[bt_i], al_s, wh, start=False, stop=stop)

        epilogue(1)
        # quarter-triggers, 2 per queue, each gated only on its own copy
        nc.sync.dma_start(out=z_d[:, 2048:2560], in_=zf_p[1][:, 0:512])
        nc.scalar.dma_start(out=z_d[:, 2560:3072], in_=zf_p[1][:, 512:1024])
        nc.sync.dma_start(out=z_d[:, 3072:3584], in_=zf_p[1][:, 1024:1536])
        nc.scalar.dma_start(out=z_d[:, 3584:4096], in_=zf_p[1][:, 1536:2048])

    nc.finalize()
    _built = nc
    return nc


def kernel(state, inputX, inputY, truth, W_t, b_t, W_x, b_x, W_y, b_y):
    global LAST_RESULT
    bf16 = ml_dtypes.bfloat16
    f32 = np.float32

    A = np.concatenate([np.asarray(state, f32), np.asarray(inputX, f32)], axis=1)
    Ah = A.astype(bf16)
    Al = (A - Ah.astype(f32)).astype(bf16)

    Wcat = np.concatenate([np.asarray(W_t, f32), np.asarray(W_x, f32)], axis=1)
    Wh = Wcat.astype(bf16)
    Wl = (Wcat - Wh.astype(f32)).astype(bf16)
    WhT = np.ascontiguousarray(Wh.T)
    WlT = np.ascontiguousarray(Wl.T)

    bias = np.asarray(b_t, f32) + np.asarray(b_x, f32)

    q = np.argmax(np.asarray(inputY), axis=1)
    Wy_sel = np.asarray(W_y, f32)[q]        # [B, H]
    by_sel = np.asarray(b_y, f32)[q]        # [B]
    truth_f = np.asarray(truth, np.float64)

    in_maps = []
    for c in range(NCORES):
        sl = slice(c * BS, (c + 1) * BS)
        in_maps.append(
            {
                "aht": np.ascontiguousarray(Ah[sl].T),
                "alt": np.ascontiguousarray(Al[sl].T),
                "wht": WhT,
                "wlt": WlT,
            }
        )

    nc = _build()
    res = run_bass_kernel_spmd(nc, in_maps, core_ids=list(range(NCORES)))
    LAST_RESULT = res

    hidden = np.empty((B, H), f32)
    for c, out in enumerate(res.results):
        sl = slice(c * BS, (c + 1) * BS)
        zc = (
            np.asarray(out["z_out"])
            .reshape(128, HH, BT, 512)
            .transpose(2, 0, 1, 3)
            .reshape(BS, H)
        )
        hidden[sl] = np.tanh(zc + bias)

    zy = np.einsum(
        "bh,bh->b", hidden.astype(np.float64), Wy_sel.astype(np.float64)
    ) + by_sel.astype(np.float64)
    pred = (1.0 / (1.0 + np.exp(-zy))).astype(f32)
    p64 = pred.astype(np.float64)
    lp = np.maximum(np.log(p64), -100.0)
    l1 = np.maximum(np.log1p(-p64), -100.0)
    bce = truth_f * lp + (1.0 - truth_f) * l1
    err = f32(-bce.sum())
    return pred, err, hidden


# revision 24
# speedup vs baseline: 1.1783x; 1.1783x over previous
import sys

sys.path.insert(0, "/opt/trn_rl_repo")

import numpy as np
import ml_dtypes

import concourse.bass as bass
import concourse.tile as tile
import concourse.mybir as mybir
from concourse import bacc
from concourse.bass_utils import run_bass_kernel_spmd

B, H, NI, NQ = 4096, 1024, 4096, 2048
NCORES = 8
BS = B // NCORES      # 512 batch rows per core
K = H + NI            # 5120 contraction dim
KT = K // 128         # 40 k-tiles
BT = BS // 128        # 4 batch subtiles per core
HH = H // 512         # 2 h-half passes

F32 = mybir.dt.float32
BF16 = mybir.dt.bfloat16
OP = mybir.AluOpType

_built = None
LAST_RESULT = None


def _build():
    global _built
    if _built is not None:
        return _built

    nc = bacc.Bacc()
    aht_d = nc.dram_tensor("aht", [K, BS], BF16, kind="ExternalInput")
    alt_d = nc.dram_tensor("alt", [K, BS], BF16, kind="ExternalInput")
    wht_d = nc.dram_tensor("wht", [K, H], BF16, kind="ExternalInput")
    wlt_d = nc.dram_tensor("wlt", [K, H], BF16, kind="ExternalInput")
    # z stored SBUF-layout: z_d[p, pass*2048 + bt*512 + j] = z[bt*128+p, pass*512+j]
    z_d = nc.dram_tensor("z_out", [128, HH * BT * 512], F32, kind="ExternalOutput")

    from contextlib import ExitStack

    with tile.TileContext(nc) as tc, ExitStack() as ctx:
        wpool = ctx.enter_context(tc.tile_pool(name="wpool", bufs=4))
        apool = ctx.enter_context(tc.tile_pool(name="apool", bufs=4))
        epool = ctx.enter_context(tc.tile_pool(name="epool", bufs=1))
        pspool = ctx.enter_context(tc.tile_pool(name="pspool", bufs=2, space="PSUM"))

        psums_p = {}
        zf_p = {}

        def epilogue(p):
            psums = psums_p[p]
            zf = epool.tile([128, BT * 512], F32, tag=f"zf{p}", name=f"zf{p}")
            zf_p[p] = zf
            # PSUM->SBUF copies split ACT{0,2} / Vector{1,3} so the last
            # bank's copy starts the moment its stop-matmul retires
            nc.scalar.copy(zf[:, 0:512], psums[0])
            nc.vector.tensor_scalar_add(zf[:, 512:1024], psums[1], 0.0)
            nc.scalar.copy(zf[:, 1024:1536], psums[2])
            nc.vector.tensor_scalar_add(zf[:, 1536:2048], psums[3], 0.0)

        for p in range(HH):
            hs = slice(p * 512, (p + 1) * 512)
            psums = [
                pspool.tile([128, 512], F32, tag=f"ps{i}", name=f"ps{p}_{i}")
                for i in range(BT)
            ]
            psums_p[p] = psums
            for kc in range(KT):
                ks = slice(kc * 128, (kc + 1) * 128)
                # W halves stream on SP queue, A tiles on ACT queue
                wh = wpool.tile([128, 512], BF16, tag="wh", name="wh")
                nc.sync.dma_start(out=wh, in_=wht_d[ks, hs])
                ah = apool.tile([128, BS], BF16, tag="ah", name="ah")
                nc.scalar.dma_start(out=ah, in_=aht_d[ks, :])
                wl = wpool.tile([128, 512], BF16, tag="wl", name="wl")
                nc.sync.dma_start(out=wl, in_=wlt_d[ks, hs])
                al = apool.tile([128, BS], BF16, tag="al", name="al")
                nc.scalar.dma_start(out=al, in_=alt_d[ks, :])

                if p == 1:
                    # pass-0 epilogue + writeback hidden under pass-1 GEMM
                    if kc == 1:
                        epilogue(0)
                    if kc == 3:
                        nc.sync.dma_start(out=z_d[:, 0:2048], in_=zf_p[0])

                start = kc == 0
                stop = kc == KT - 1
                for bt_i in range(BT):
                    ah_s = ah[:, bt_i * 128 : (bt_i + 1) * 128]
                    al_s = al[:, bt_i * 128 : (bt_i + 1) * 128]
                    nc.tensor.matmul(psums[bt_i], ah_s, wh, start=start, stop=False)
                    nc.tensor.matmul(psums[bt_i], ah_s, wl, start=False, stop=False)
                    nc.tensor.matmul(psums[bt_i], al_s, wh, start=False, stop=stop)

        epilogue(1)
        # split across both DMA queues so the final drain halves
        nc.sync.dma_start(out=z_d[:, 2048:3072], in_=zf_p[1][:, 0:1024])
        nc.scalar.dma_start(out=z_d[:, 3072:4096], in_=zf_p[1][:, 1024:2048])

    nc.finalize()
    _built = nc
    return nc


def kernel(state, inputX, inputY, truth, W_t, b_t, W_x, b_x, W_y, b_y):
    global LAST_RESULT
    bf16 = ml_dtypes.bfloat16
    f32 = np.float32

    A = np.concatenate([np.asarray(state, f32), np.asarray(inputX, f32)], axis=1)
    Ah = A.astype(bf16)
    Al = (A - Ah.astype(f32)).astype(bf16)

    Wcat = np.concatenate([np.asarray(W_t, f32), np.asarray(W_x, f32)], axis=1)
    Wh = Wcat.astype(bf16)
    Wl = (Wcat - Wh.astype(f32)).astype(bf16)
    WhT = np.ascontiguousarray(Wh.T)
    WlT = np.ascontiguousarray(Wl.T)

    bias = np.asarray(b_t, f32) + np.asarray(b_x, f32)

    q = np.argmax(np.asarray(inputY), axis=1)
    Wy_sel = np.asarray(W_y, f32)[q]        # [B, H]
    by_sel = np.asarray(b_y, f32)[q]        # [B]
    truth_f = np.asarray(truth, np.float64)

    in_maps = []
    for c in range(NCORES):
        sl = slice(c * BS, (c + 1) * BS)
        in_maps.append(
            {
                "aht": np.ascontiguousarray(Ah[sl].T),
                "alt": np.ascontiguousarray(Al[sl].T),
                "wht": WhT,
                "wlt": WlT,
            }
        )

    nc = _build()
    res = run_bass_kernel_spmd(nc, in_maps, core_ids=list(range(NCORES)))
    LAST_RESULT = res

    hidden = np.empty((B, H), f32)
    for c, out in enumerate(res.results):
        sl = slice(c * BS, (c + 1) * BS)
        zc = (
            np.asarray(out["z_out"])
            .reshape(128, HH, BT, 512)
            .transpose(2, 0, 1, 3)
            .reshape(BS, H)
        )
        hidden[sl] = np.tanh(zc + bias)

    zy = np.einsum(
        "bh,bh->b", hidden.astype(np.float64), Wy_sel.astype(np.float64)
    ) + by_sel.astype(np.float64)
    pred = (1.0 / (1.0 + np.exp(-zy))).astype(f32)
    p64 = pred.astype(np.float64)
    lp = np.maximum(np.log(p64), -100.0)
    l1 = np.maximum(np.log1p(-p64), -100.0)
    bce = truth_f * lp + (1.0 - truth_f) * l1
    err = f32(-bce.sum())
    return pred, err, hidden


# revision 25
# speedup vs baseline: 1.1933x; 1.0127x over previous
import sys

sys.path.insert(0, "/opt/trn_rl_repo")

import numpy as np
import ml_dtypes

import concourse.bass as bass
import concourse.tile as tile
import concourse.mybir as mybir
from concourse import bacc
from concourse.bass_utils import run_bass_kernel_spmd

B, H, NI, NQ = 4096, 1024, 4096, 2048
NCORES = 8
BS = B // NCORES      # 512 batch rows per core
K = H + NI            # 5120 contraction dim
KT = K // 128         # 40 k-tiles
BT = BS // 128        # 4 batch subtiles per core
HH = H // 512         # 2 h-half passes

F32 = mybir.dt.float32
BF16 = mybir.dt.bfloat16
OP = mybir.AluOpType

_built = None
LAST_RESULT = None


def _build():
    global _built
    if _built is not None:
        return _built

    nc = bacc.Bacc()
    aht_d = nc.dram_tensor("aht", [K, BS], BF16, kind="ExternalInput")
    alt_d = nc.dram_tensor("alt", [K, BS], BF16, kind="ExternalInput")
    wht_d = nc.dram_tensor("wht", [K, H], BF16, kind="ExternalInput")
    wlt_d = nc.dram_tensor("wlt", [K, H], BF16, kind="ExternalInput")
    # z stored SBUF-layout: z_d[p, pass*2048 + bt*512 + j] = z[bt*128+p, pass*512+j]
    z_d = nc.dram_tensor("z_out", [128, HH * BT * 512], F32, kind="ExternalOutput")

    from contextlib import ExitStack

    with tile.TileContext(nc) as tc, ExitStack() as ctx:
        wpool = ctx.enter_context(tc.tile_pool(name="wpool", bufs=4))
        apool = ctx.enter_context(tc.tile_pool(name="apool", bufs=4))
        epool = ctx.enter_context(tc.tile_pool(name="epool", bufs=1))
        pspool = ctx.enter_context(tc.tile_pool(name="pspool", bufs=2, space="PSUM"))

        psums_p = {}
        zf_p = {}

        def epilogue(p):
            psums = psums_p[p]
            zf = epool.tile([128, BT * 512], F32, tag=f"zf{p}", name=f"zf{p}")
            zf_p[p] = zf
            # PSUM->SBUF copies split ACT{0,2} / Vector{1,3} so the last
            # bank's copy starts the moment its stop-matmul retires
            nc.scalar.copy(zf[:, 0:512], psums[0])
            nc.vector.tensor_scalar_add(zf[:, 512:1024], psums[1], 0.0)
            nc.scalar.copy(zf[:, 1024:1536], psums[2])
            nc.vector.tensor_scalar_add(zf[:, 1536:2048], psums[3], 0.0)

        for p in range(HH):
            hs = slice(p * 512, (p + 1) * 512)
            psums = [
                pspool.tile([128, 512], F32, tag=f"ps{i}", name=f"ps{p}_{i}")
                for i in range(BT)
            ]
            psums_p[p] = psums
            for kc in range(KT):
                ks = slice(kc * 128, (kc + 1) * 128)
                # W halves stream on SP queue, A tiles on ACT queue
                wh = wpool.tile([128, 512], BF16, tag="wh", name="wh")
                nc.sync.dma_start(out=wh, in_=wht_d[ks, hs])
                ah = apool.tile([128, BS], BF16, tag="ah", name="ah")
                nc.scalar.dma_start(out=ah, in_=aht_d[ks, :])
                wl = wpool.tile([128, 512], BF16, tag="wl", name="wl")
                nc.sync.dma_start(out=wl, in_=wlt_d[ks, hs])
                al = apool.tile([128, BS], BF16, tag="al", name="al")
                nc.scalar.dma_start(out=al, in_=alt_d[ks, :])

                if p == 1:
                    # pass-0 epilogue + writeback hidden under pass-1 GEMM
                    if kc == 1:
                        epilogue(0)
                    if kc == 3:
                        nc.sync.dma_start(out=z_d[:, 0:2048], in_=zf_p[0])

                start = kc == 0
                stop = kc == KT - 1
                for bt_i in range(BT):
                    ah_s = ah[:, bt_i * 128 : (bt_i + 1) * 128]
                    al_s = al[:, bt_i * 128 : (bt_i + 1) * 128]
                    nc.tensor.matmul(psums[bt_i], ah_s, wh, start=start, stop=False)
                    nc.tensor.matmul(psums[bt_i], ah_s, wl, start=False, stop=False)
                    nc.tensor.matmul(psums[bt_i], al_s, wh, start=False, stop=stop)

        epilogue(1)
        # quarter-triggers, 2 per queue, each gated only on its own copy
        nc.sync.dma_start(out=z_d[:, 2048:2560], in_=zf_p[1][:, 0:512])
        nc.scalar.dma_start(out=z_d[:, 2560:3072], in_=zf_p[1][:, 512:1024])
        nc.sync.dma_start(out=z_d[:, 3072:3584], in_=zf_p[1][:, 1024:1536])
        nc.scalar.dma_start(out=z_d[:, 3584:4096], in_=zf_p[1][:, 1536:2048])

    nc.finalize()
    _built = nc
    return nc


def kernel(state, inputX, inputY, truth, W_t, b_t, W_x, b_x, W_y, b_y):
    global LAST_RESULT
    bf16 = ml_dtypes.bfloat16
    f32 = np.float32

    A = np.concatenate([np.asarray(state, f32), np.asarray(inputX, f32)], axis=1)
    Ah = A.astype(bf16)
    Al = (A - Ah.astype(f32)).astype(bf16)

    Wcat = np.concatenate([np.asarray(W_t, f32), np.asarray(W_x, f32)], axis=1)
    Wh = Wcat.astype(bf16)
    Wl = (Wcat - Wh.astype(f32)).astype(bf16)
    WhT = np.ascontiguousarray(Wh.T)
    WlT = np.ascontiguousarray(Wl.T)

    bias = np.asarray(b_t, f32) + np.asarray(b_x, f32)

    q = np.argmax(np.asarray(inputY), axis=1)
    Wy_sel = np.asarray(W_y, f32)[q]        # [B, H]
    by_sel = np.asarray(b_y, f32)[q]        # [B]
    truth_f = np.asarray(truth, np.float64)

    in_maps = []
    for c in range(NCORES):
        sl = slice(c * BS, (c + 1) * BS)
        in_maps.append(
            {
                "aht": np.ascontiguousarray(Ah[sl].T),
                "alt": np.ascontiguousarray(Al[sl].T),
                "wht": WhT,
                "wlt": WlT,
            }
        )

    nc = _build()
    res = run_bass_kernel_spmd(nc, in_maps, core_ids=list(range(NCORES)))
    LAST_RESULT = res

    hidden = np.empty((B, H), f32)
    for c, out in enumerate(res.results):
        sl = slice(c * BS, (c + 1) * BS)
        zc = (
            np.asarray(out["z_out"])
            .reshape(128, HH, BT, 512)
            .transpose(2, 0, 1, 3)
            .reshape(BS, H)
        )
        hidden[sl] = np.tanh(zc + bias)

    zy = np.einsum(
        "bh,bh->b", hidden.astype(np.float64), Wy_sel.astype(np.float64)
    ) + by_sel.astype(np.float64)
    pred = (1.0 / (1.0 + np.exp(-zy))).astype(f32)
    p64 = pred.astype(np.float64)
    lp = np.maximum(np.log(p64), -100.0)
    l1 = np.maximum(np.log1p(-p64), -100.0)
    bce = truth_f * lp + (1.0 - truth_f) * l1
    err = f32(-bce.sum())
    return pred, err, hidden
